# revision 17
# baseline (speedup 1.0000x reference)
"""Multi-head self-attention (B=4, S=2048, E=1024, H=16, causal) on 8 TRN2
NeuronCores, tensor-parallel over heads (2 heads/core).

Per-core pipeline (all matmuls bf16, fp32 PSUM accumulation):
  1. QKV projection from a host-transposed query qT [E, T]:
       Q^T,K^T [128(2h*64d), T] via lhsT=w^T chunks; V [t,128] natural layout.
     b_q/b_k folded as per-partition ACT biases; b_v folded into the output
     bias on the host (b_out_eff = b_out + w_out @ b_v).
  2. Causal attention in S^T layout (keys on partitions, queries on free dim):
       S^T[k,q] = K^T.T @ Q^T ; P = exp(S/8) on ScalarE (no max subtraction:
       inputs are unit-scale gaussians, scores ~ N(0,1));
       PV via lhsT=V_aug (ones column appended -> row 64 = softmax sums).
     Causal: blocks with k > q skipped; diagonal blocks masked post-exp.
     Normalization deferred past the A2A. attnU row 64 carries the sums.
  3. Two half-AllToAlls (even/odd 512-token q-blocks) so the re-shard +
     normalize + output projection of the first half overlaps the attention
     of the second; per-token reciprocal + PE-broadcast normalization,
     output projection, out^T [E, 1024] per core; host concatenates.
"""
import sys

if "/opt/trn_rl_repo" not in sys.path:
    sys.path.insert(0, "/opt/trn_rl_repo")

import numpy as np
import ml_dtypes

BF16 = ml_dtypes.bfloat16

B, S, E, H, D = 4, 2048, 1024, 16, 64
T = B * S  # 8192
N_CORES = 8
HPC = H // N_CORES  # 2 heads per core
TL = T // N_CORES  # 1024 tokens per core for the output shard
NTB = T // 512  # 16 projection t-blocks
SCALE = 1.0 / np.sqrt(D)

_CACHE = {}


def build_kernel():
    import concourse.mybir as mybir
    import concourse.tile as tile
    from concourse import bacc
    from concourse.bass import ds, ts, _add_dep_helper

    F32 = mybir.dt.float32
    BF = mybir.dt.bfloat16
    AF = mybir.ActivationFunctionType
    ALU = mybir.AluOpType

    nc = bacc.Bacc("TRN2", target_bir_lowering=False, debug=False,
                   num_devices=N_CORES)

    qT_d = nc.dram_tensor("qT", [E, T], BF, kind="ExternalInput")
    wqk_d = nc.dram_tensor("wqk", [E, 256], BF, kind="ExternalInput")
    wv_d = nc.dram_tensor("wv", [E, 128], BF, kind="ExternalInput")
    bqk_d = nc.dram_tensor("bqk", [128, 2], F32, kind="ExternalInput")
    wout_d = nc.dram_tensor("wout", [E, E], BF, kind="ExternalInput")
    bout_d = nc.dram_tensor("bout", [128, 8], F32, kind="ExternalInput")
    masks_d = nc.dram_tensor("masks", [128, 4, 512], BF, kind="ExternalInput")
    sel_d = nc.dram_tensor("sel", [16, 8, 128], BF, kind="ExternalInput")
    outT_d = nc.dram_tensor("outT", [E, TL], F32, kind="ExternalOutput")

    with tile.TileContext(nc) as tc:
        with (
            tc.tile_pool(name="consts", bufs=1) as cpool,
            tc.tile_pool(name="dram", bufs=1, space="DRAM") as dram,
            tc.tile_pool(name="spair", bufs=2, space="PSUM") as ps_pair,
            tc.tile_pool(name="att", bufs=2, space="PSUM") as ps_att,
            tc.tile_pool(name="psv", bufs=2, space="PSUM") as ps_v,
            tc.tile_pool(name="persist", bufs=1) as ppool,
            tc.tile_pool(name="qt", bufs=2) as qtpool,
            tc.tile_pool(name="pex", bufs=4) as ppex,
            tc.tile_pool(name="ph3", bufs=2) as p3,
        ):
            # ---- constants to SBUF
            wqk_sb = cpool.tile([128, 8, 256], BF)
            nc.sync.dma_start(wqk_sb[:], wqk_d.ap().rearrange("(c p) f -> p c f", p=128))
            wv_sb = cpool.tile([128, 8, 128], BF)
            nc.sync.dma_start(wv_sb[:], wv_d.ap().rearrange("(c p) f -> p c f", p=128))
            bqk_sb = cpool.tile([128, 2], F32)
            nc.sync.dma_start(bqk_sb[:], bqk_d.ap())
            wout_sb = cpool.tile([128, 8, 1024], BF)
            nc.sync.dma_start(wout_sb[:], wout_d.ap().rearrange("(c p) e -> p c e", p=128))
            bout_sb = cpool.tile([128, 8], F32)
            nc.sync.dma_start(bout_sb[:], bout_d.ap())
            masks_sb = cpool.tile([128, 4, 512], BF)
            nc.sync.dma_start(masks_sb[:], masks_d.ap())
            sel_sb = cpool.tile([16, 8, 128], BF)
            nc.sync.dma_start(sel_sb[:], sel_d.ap())

            q_sb = ppool.tile([128, T], BF, tag="q_sb")
            k_sb = ppool.tile([128, T], BF, tag="k_sb")
            v_sb = ppool.tile([128, 64, 131], BF, tag="v_sb")
            # rows 0-63: unnormalized attn^T per head; row 64: softmax sums
            attnU = ppool.tile([65, 2, T], BF, tag="attnU")

            nc.vector.memset(v_sb[:, :, 64:65], 1.0)
            nc.vector.memset(v_sb[:, :, 130:131], 1.0)

            # ---- phase 1: QKV projection over 512-token blocks
            qT_r = qT_d.ap().rearrange("(c p) t -> p c t", p=128)

            def emit_proj(tb):
                qt = qtpool.tile([128, 8, 512], BF, name="qt")
                nc.sync.dma_start(qt[:], qT_r[:, :, ts(tb, 512)])
                ps = ps_pair.tile([128, 1024], F32, tag="sp", name="ps")
                for c in range(8):
                    nc.tensor.matmul(ps[:, 0:512], wqk_sb[:, c, 0:128],
                                     qt[:, c, :], start=(c == 0), stop=(c == 7))
                for c in range(8):
                    nc.tensor.matmul(ps[:, 512:1024], wqk_sb[:, c, 128:256],
                                     qt[:, c, :], start=(c == 0), stop=(c == 7))
                nc.scalar.activation(q_sb[:, ts(tb, 512)], ps[:, 0:512],
                                     AF.Identity, bias=bqk_sb[:, 0:1])
                nc.scalar.activation(k_sb[:, ts(tb, 512)], ps[:, 512:1024],
                                     AF.Identity, bias=bqk_sb[:, 1:2])
                for sub in range(4):
                    t128 = tb * 4 + sub
                    psv = ps_v.tile([128, 128], F32, tag="psv", name="psv")
                    for c in range(8):
                        nc.tensor.matmul(psv[:], qt[:, c, ds(sub * 128, 128)],
                                         wv_sb[:, c, :], start=(c == 0), stop=(c == 7))
                    nc.vector.tensor_copy(v_sb[:, t128, 0:64], psv[:, 0:64])
                    nc.vector.tensor_copy(v_sb[:, t128, 66:130], psv[:, 64:128])

            # ---- phase 2+3 interleaved by halves (even q-blocks, odd q-blocks)
            def attention_unit(b, j, h):
                q0 = b * S + j * 512
                hp = h * 64
                vlo = 0 if h == 0 else 66
                att = ps_att.tile([65, 512], F32, tag="att")
                nkb = 4 * j + 4
                for g in range(2 * j + 2):  # groups of 2 k-blocks
                    sp = ps_pair.tile([128, 1024], F32, tag="sp")
                    for u in range(2):
                        kb = 2 * g + u
                        k0 = b * S + kb * 128
                        # diagonal blocks: only q >= 128m is causally valid;
                        # the stale psum left of it is zeroed by the mask
                        off = max(0, (kb - 4 * j) * 128)
                        nc.tensor.matmul(
                            sp[:, ds(u * 512 + off, 512 - off)],
                            k_sb[ds(hp, 64), ds(k0, 128)],
                            q_sb[ds(hp, 64), ds(q0 + off, 512 - off)],
                            start=True, stop=True)
                    p = ppex.tile([128, 1024], BF)
                    last_exp = nc.scalar.activation(p[:], sp[:], AF.Exp, scale=SCALE)
                    for u in range(2):
                        kb = 2 * g + u
                        m = kb - 4 * j
                        if m >= 0:  # diagonal block: causal mask
                            nc.vector.tensor_tensor(
                                p[:, ds(u * 512, 512)], p[:, ds(u * 512, 512)],
                                masks_sb[:, m, :], op=ALU.mult)
                    for u in range(2):
                        kb = 2 * g + u
                        t128 = b * 16 + kb
                        last_pv = nc.tensor.matmul(
                            att[:], v_sb[:, t128, ds(vlo, 65)],
                            p[:, ds(u * 512, 512)],
                            start=(kb == 0), stop=(kb == nkb - 1))
                epi = nc.vector.tensor_copy(attnU[:, h, ds(q0, 512)], att[:])
                return last_exp, last_pv, epi

            a2a_in = [dram.tile([N_CORES, 130, 512], BF, tag=f"a2a_in{i}",
                                name=f"a2a_in{i}") for i in range(2)]
            a2a_out = [dram.tile([N_CORES, 130, 512], BF, tag=f"a2a_out{i}",
                                 name=f"a2a_out{i}") for i in range(2)]

            def stage_and_a2a(half):
                src = attnU[:, :, :].rearrange("p h (c t) -> p h c t", c=N_CORES)
                sl = ds(half * 512, 512)
                nc.sync.dma_start(
                    a2a_in[half][:, 0:64, :].rearrange("c p t -> p c t"),
                    src[0:64, 0, :, sl])
                nc.sync.dma_start(
                    a2a_in[half][:, 64:128, :].rearrange("c p t -> p c t"),
                    src[0:64, 1, :, sl])
                nc.sync.dma_start(
                    a2a_in[half][:, 128:130, :].rearrange("c h t -> h c t"),
                    src[64:65, :, :, sl])
                nc.gpsimd.collective_compute(
                    "AllToAll", ALU.bypass,
                    replica_groups=[list(range(N_CORES))],
                    ins=[a2a_in[half][:].opt()], outs=[a2a_out[half][:].opt()])

            def phase3_prefetch(half):
                af = p3.tile([128, 8, 512], BF, tag="af", name="af")
                rsrc = p3.tile([16, 512], BF, tag="rsrc", name="rsrc")
                nc.sync.dma_start(
                    af[:], a2a_out[half][:, 0:128, :].rearrange("c p t -> p c t"))
                nc.sync.dma_start(rsrc[:], a2a_out[half][:, 128:130, :])
                return af, rsrc

            def phase3_compute(half, af, rsrc, gates):
                """gates: dict engine->BassInstruction the first op of that
                engine's queue must not be scheduled before."""
                def gate(inst, eng):
                    if gates.get(eng) is not None:
                        _add_dep_helper(inst.ins, gates[eng].ins, sync=False,
                                        reason="phase3 queue-order gate")
                    gates[eng] = None

                rf32 = p3.tile([16, 512], F32, tag="rf32", name="rf32")
                rbf = p3.tile([16, 512], BF, tag="rbf", name="rbf")
                gate(nc.vector.reciprocal(rf32[:], rsrc[:]), "v")
                nc.vector.tensor_copy(rbf[:], rf32[:])
                last_tt = None
                for c in range(8):
                    rb = ps_att.tile([128, 512], F32, tag="att", name="rb")
                    gate(nc.tensor.matmul(rb[:], sel_sb[:, c, :], rbf[:],
                                          start=True, stop=True), "pe")
                    last_tt = nc.vector.tensor_tensor(af[:, c, :], af[:, c, :],
                                                      rb[:], op=ALU.mult)
                osb = p3.tile([128, 8, 512], F32, tag="osb", name="osb")
                last_act = last_mm = None
                for m in range(8):
                    po = ps_v.tile([128, 512], F32, tag="psv", name="po")
                    for c in range(8):
                        last_mm = nc.tensor.matmul(
                            po[:], wout_sb[:, c, ds(m * 128, 128)],
                            af[:, c, :], start=(c == 0), stop=(c == 7))
                    last_act = nc.scalar.activation(osb[:, m, :], po[:], AF.Identity,
                                                    bias=bout_sb[:, ds(m, 1)])
                    if m == 0:
                        gate(last_act, "s")
                nc.sync.dma_start(
                    outT_d.ap().rearrange("(m p) t -> p m t", p=128)[:, :, ts(half, 512)],
                    osb[:])
                return {"v": last_tt, "s": last_act, "pe": last_mm}

            # Emission order = engine-queue order. Interleave projection
            # t-blocks (PE-heavy) with even-q-block attention (ACT-heavy);
            # A2A half 0 overlaps odd-q-block attention; phase 3 of half 0
            # overlaps the A2A of half 1.
            for tb in range(4):
                emit_proj(tb)
            pi = 4
            for b in range(B):
                for j in (0, 2):
                    for h in range(HPC):
                        attention_unit(b, j, h)
                        if pi < NTB:
                            emit_proj(pi)
                            pi += 1
            stage_and_a2a(0)
            af0, rsrc0 = phase3_prefetch(0)  # loads run mid-pass-2, after cc0
            for b in range(B):
                for j in (1, 3):
                    for h in range(HPC):
                        le, lp, lc = attention_unit(b, j, h)
            stage_and_a2a(1)
            af1, rsrc1 = phase3_prefetch(1)
            lasts = phase3_compute(0, af0, rsrc0, {"v": lc, "s": le, "pe": lp})
            phase3_compute(1, af1, rsrc1, lasts)

    nc.compile()
    return nc


def prep_inputs(query, w_in, b_in, w_out, b_out):
    """Shard + lay out host-side. Returns in_maps for the 8 cores."""
    query = np.asarray(query, dtype=np.float32)
    w_in = np.asarray(w_in, dtype=np.float32)
    b_in = np.asarray(b_in, dtype=np.float32)
    w_out = np.asarray(w_out, dtype=np.float32)
    b_out = np.asarray(b_out, dtype=np.float32)

    qT = np.ascontiguousarray(query.reshape(T, E).T).astype(BF16)
    woutT = np.ascontiguousarray(w_out.T).astype(BF16)
    b_v = b_in[2 * E:3 * E]
    bout_eff = (b_out + w_out @ b_v).reshape(8, 128).T.copy()  # [128, 8]

    # causal masks for the 4 diagonal 128x512 blocks: mask[m][p, q] = p <= q-128m
    qidx = np.arange(512)[None, :]
    pidx = np.arange(128)[:, None]
    masks = np.stack([(pidx <= qidx - 128 * m) for m in range(4)], axis=1)
    masks = masks.astype(BF16)  # [128, 4, 512]

    sel = np.zeros((16, 8, 128), dtype=BF16)
    for c in range(8):
        sel[2 * c, c, 0:64] = 1.0
        sel[2 * c + 1, c, 64:128] = 1.0

    in_maps = []
    for c in range(N_CORES):
        r = slice(128 * c, 128 * c + 128)
        wqk = np.concatenate([w_in[:E][r].T, w_in[E:2 * E][r].T], axis=1)
        wv = w_in[2 * E:3 * E][r].T
        bqk = np.stack([b_in[:E][r], b_in[E:2 * E][r]], axis=1)
        in_maps.append({
            "qT": qT,
            "wqk": np.ascontiguousarray(wqk).astype(BF16),
            "wv": np.ascontiguousarray(wv).astype(BF16),
            "bqk": np.ascontiguousarray(bqk),
            "wout": woutT,
            "bout": np.ascontiguousarray(bout_eff),
            "masks": masks,
            "sel": sel,
        })
    return in_maps


def run_on_hw(in_maps, trace=False, **kw):
    from concourse.bass_utils import run_bass_kernel_spmd

    if "nc" not in _CACHE:
        _CACHE["nc"] = build_kernel()
    return run_bass_kernel_spmd(_CACHE["nc"], in_maps, list(range(N_CORES)),
                                trace=trace, **kw)


def kernel(query, w_in, b_in, w_out, b_out):
    in_maps = prep_inputs(query, w_in, b_in, w_out, b_out)
    res = run_on_hw(in_maps)
    parts = [res.results[c]["outT"].T for c in range(N_CORES)]  # [TL, E] each
    out = np.concatenate(parts, axis=0).reshape(B, S, E)
    return out.astype(np.float32)


# revision 18
# speedup vs baseline: 1.0849x; 1.0849x over previous
"""Multi-head self-attention (B=4, S=2048, E=1024, H=16, causal) on 8 TRN2
NeuronCores, tensor-parallel over heads (2 heads/core).

Per-core pipeline (all matmuls bf16, fp32 PSUM accumulation):
  1. QKV projection from a host-transposed query qT [E, T]:
       Q^T,K^T [128(2h*64d), T] via lhsT=w^T chunks; V [t,128] natural layout.
     b_q/b_k folded as per-partition ACT biases; b_v folded into the output
     bias on the host (b_out_eff = b_out + w_out @ b_v).
  2. Causal attention in S^T layout (keys on partitions, queries on free dim):
       S^T[k,q] = K^T.T @ Q^T ; P = exp(S/8) on ScalarE (no max subtraction:
       inputs are unit-scale gaussians, scores ~ N(0,1));
       PV via lhsT=V_aug (ones column appended -> row 64 = softmax sums).
     Causal: blocks with k > q skipped; diagonal blocks masked post-exp.
     Normalization deferred past the A2A. attnU row 64 carries the sums.
  3. Two half-AllToAlls (even/odd 512-token q-blocks) so the re-shard +
     normalize + output projection of the first half overlaps the attention
     of the second; per-token reciprocal + PE-broadcast normalization,
     output projection, out^T [E, 1024] per core; host concatenates.
"""
import sys

if "/opt/trn_rl_repo" not in sys.path:
    sys.path.insert(0, "/opt/trn_rl_repo")

import numpy as np
import ml_dtypes

BF16 = ml_dtypes.bfloat16

B, S, E, H, D = 4, 2048, 1024, 16, 64
T = B * S  # 8192
N_CORES = 8
HPC = H // N_CORES  # 2 heads per core
TL = T // N_CORES  # 1024 tokens per core for the output shard
NTB = T // 512  # 16 projection t-blocks
SCALE = 1.0 / np.sqrt(D)

_CACHE = {}


def build_kernel():
    import concourse.mybir as mybir
    import concourse.tile as tile
    from concourse import bacc
    from concourse.bass import ds, ts, _add_dep_helper

    F32 = mybir.dt.float32
    BF = mybir.dt.bfloat16
    AF = mybir.ActivationFunctionType
    ALU = mybir.AluOpType

    nc = bacc.Bacc("TRN2", target_bir_lowering=False, debug=False,
                   num_devices=N_CORES)

    qT_d = nc.dram_tensor("qT", [E, T], BF, kind="ExternalInput")
    wqk_d = nc.dram_tensor("wqk", [E, 256], BF, kind="ExternalInput")
    wv_d = nc.dram_tensor("wv", [E, 128], BF, kind="ExternalInput")
    bqk_d = nc.dram_tensor("bqk", [128, 2], F32, kind="ExternalInput")
    wout_d = nc.dram_tensor("wout", [E, E], BF, kind="ExternalInput")
    bout_d = nc.dram_tensor("bout", [128, 8], F32, kind="ExternalInput")
    masks_d = nc.dram_tensor("masks", [128, 4, 512], BF, kind="ExternalInput")
    sel_d = nc.dram_tensor("sel", [16, 8, 128], BF, kind="ExternalInput")
    outT_d = nc.dram_tensor("outT", [E, TL], F32, kind="ExternalOutput")

    with tile.TileContext(nc) as tc:
        with (
            tc.tile_pool(name="consts", bufs=1) as cpool,
            tc.tile_pool(name="dram", bufs=1, space="DRAM") as dram,
            tc.tile_pool(name="spair", bufs=2, space="PSUM") as ps_pair,
            tc.tile_pool(name="att", bufs=2, space="PSUM") as ps_att,
            tc.tile_pool(name="psv", bufs=2, space="PSUM") as ps_v,
            tc.tile_pool(name="persist", bufs=1) as ppool,
            tc.tile_pool(name="qt", bufs=2) as qtpool,
            tc.tile_pool(name="pex", bufs=4) as ppex,
            tc.tile_pool(name="ph3", bufs=2) as p3,
        ):
            # ---- constants to SBUF
            wqk_sb = cpool.tile([128, 8, 256], BF)
            nc.sync.dma_start(wqk_sb[:], wqk_d.ap().rearrange("(c p) f -> p c f", p=128))
            wv_sb = cpool.tile([128, 8, 128], BF)
            nc.sync.dma_start(wv_sb[:], wv_d.ap().rearrange("(c p) f -> p c f", p=128))
            bqk_sb = cpool.tile([128, 2], F32)
            nc.sync.dma_start(bqk_sb[:], bqk_d.ap())
            wout_sb = cpool.tile([128, 8, 1024], BF)
            nc.sync.dma_start(wout_sb[:], wout_d.ap().rearrange("(c p) e -> p c e", p=128))
            bout_sb = cpool.tile([128, 8], F32)
            nc.sync.dma_start(bout_sb[:], bout_d.ap())
            masks_sb = cpool.tile([128, 4, 512], BF)
            nc.sync.dma_start(masks_sb[:], masks_d.ap())
            sel_sb = cpool.tile([16, 8, 128], BF)
            nc.sync.dma_start(sel_sb[:], sel_d.ap())

            q_sb = ppool.tile([128, T], BF, tag="q_sb")
            k_sb = ppool.tile([128, T], BF, tag="k_sb")
            v_sb = ppool.tile([128, 64, 131], BF, tag="v_sb")
            # rows 0-63: unnormalized attn^T per head; row 64: softmax sums
            attnU = ppool.tile([65, 2, T], BF, tag="attnU")

            nc.vector.memset(v_sb[:, :, 64:65], 1.0)
            nc.vector.memset(v_sb[:, :, 130:131], 1.0)

            # ---- phase 1: QKV projection over 512-token blocks
            qT_r = qT_d.ap().rearrange("(c p) t -> p c t", p=128)

            def emit_proj(tb):
                qt = qtpool.tile([128, 8, 512], BF, name="qt")
                nc.sync.dma_start(qt[:], qT_r[:, :, ts(tb, 512)])
                ps = ps_pair.tile([128, 1024], F32, tag="sp", name="ps")
                for c in range(8):
                    nc.tensor.matmul(ps[:, 0:512], wqk_sb[:, c, 0:128],
                                     qt[:, c, :], start=(c == 0), stop=(c == 7))
                for c in range(8):
                    nc.tensor.matmul(ps[:, 512:1024], wqk_sb[:, c, 128:256],
                                     qt[:, c, :], start=(c == 0), stop=(c == 7))
                nc.scalar.activation(q_sb[:, ts(tb, 512)], ps[:, 0:512],
                                     AF.Identity, bias=bqk_sb[:, 0:1])
                nc.scalar.activation(k_sb[:, ts(tb, 512)], ps[:, 512:1024],
                                     AF.Identity, bias=bqk_sb[:, 1:2])
                for sub in range(4):
                    t128 = tb * 4 + sub
                    psv = ps_v.tile([128, 128], F32, tag="psv", name="psv")
                    for c in range(8):
                        nc.tensor.matmul(psv[:], qt[:, c, ds(sub * 128, 128)],
                                         wv_sb[:, c, :], start=(c == 0), stop=(c == 7))
                    nc.vector.tensor_copy(v_sb[:, t128, 0:64], psv[:, 0:64])
                    nc.vector.tensor_copy(v_sb[:, t128, 66:130], psv[:, 64:128])

            # ---- phase 2+3 interleaved by halves (even q-blocks, odd q-blocks)
            def attention_unit(b, j, h):
                q0 = b * S + j * 512
                hp = h * 64
                vlo = 0 if h == 0 else 66
                att = ps_att.tile([65, 512], F32, tag="att")
                nkb = 4 * j + 4
                for g in range(2 * j + 2):  # groups of 2 k-blocks
                    sp = ps_pair.tile([128, 1024], F32, tag="sp")
                    for u in range(2):
                        kb = 2 * g + u
                        k0 = b * S + kb * 128
                        nc.tensor.matmul(
                            sp[:, ds(u * 512, 512)],
                            k_sb[ds(hp, 64), ds(k0, 128)],
                            q_sb[ds(hp, 64), ds(q0, 512)],
                            start=True, stop=True)
                    p = ppex.tile([128, 1024], BF)
                    last_exp = nc.scalar.activation(p[:], sp[:], AF.Exp, scale=SCALE)
                    for u in range(2):
                        kb = 2 * g + u
                        m = kb - 4 * j
                        if m >= 0:  # diagonal block: causal mask
                            nc.vector.tensor_tensor(
                                p[:, ds(u * 512, 512)], p[:, ds(u * 512, 512)],
                                masks_sb[:, m, :], op=ALU.mult)
                    for u in range(2):
                        kb = 2 * g + u
                        t128 = b * 16 + kb
                        last_pv = nc.tensor.matmul(
                            att[:], v_sb[:, t128, ds(vlo, 65)],
                            p[:, ds(u * 512, 512)],
                            start=(kb == 0), stop=(kb == nkb - 1))
                epi = nc.vector.tensor_copy(attnU[:, h, ds(q0, 512)], att[:])
                return last_exp, last_pv, epi

            a2a_in = [dram.tile([N_CORES, 130, 512], BF, tag=f"a2a_in{i}",
                                name=f"a2a_in{i}") for i in range(2)]
            a2a_out = [dram.tile([N_CORES, 130, 512], BF, tag=f"a2a_out{i}",
                                 name=f"a2a_out{i}") for i in range(2)]

            def stage_and_a2a(half):
                src = attnU[:, :, :].rearrange("p h (c t) -> p h c t", c=N_CORES)
                sl = ds(half * 512, 512)
                nc.sync.dma_start(
                    a2a_in[half][:, 0:64, :].rearrange("c p t -> p c t"),
                    src[0:64, 0, :, sl])
                nc.sync.dma_start(
                    a2a_in[half][:, 64:128, :].rearrange("c p t -> p c t"),
                    src[0:64, 1, :, sl])
                nc.sync.dma_start(
                    a2a_in[half][:, 128:130, :].rearrange("c h t -> h c t"),
                    src[64:65, :, :, sl])
                nc.gpsimd.collective_compute(
                    "AllToAll", ALU.bypass,
                    replica_groups=[list(range(N_CORES))],
                    ins=[a2a_in[half][:].opt()], outs=[a2a_out[half][:].opt()])

            def phase3_prefetch(half):
                af = p3.tile([128, 8, 512], BF, tag="af", name="af")
                rsrc = p3.tile([16, 512], BF, tag="rsrc", name="rsrc")
                nc.sync.dma_start(
                    af[:], a2a_out[half][:, 0:128, :].rearrange("c p t -> p c t"))
                nc.sync.dma_start(rsrc[:], a2a_out[half][:, 128:130, :])
                return af, rsrc

            def phase3_compute(half, af, rsrc, gates):
                """gates: dict engine->BassInstruction the first op of that
                engine's queue must not be scheduled before."""
                def gate(inst, eng):
                    if gates.get(eng) is not None:
                        _add_dep_helper(inst.ins, gates[eng].ins, sync=False,
                                        reason="phase3 queue-order gate")
                    gates[eng] = None

                rf32 = p3.tile([16, 512], F32, tag="rf32", name="rf32")
                rbf = p3.tile([16, 512], BF, tag="rbf", name="rbf")
                gate(nc.vector.reciprocal(rf32[:], rsrc[:]), "v")
                nc.vector.tensor_copy(rbf[:], rf32[:])
                last_tt = None
                for c in range(8):
                    rb = ps_att.tile([128, 512], F32, tag="att", name="rb")
                    gate(nc.tensor.matmul(rb[:], sel_sb[:, c, :], rbf[:],
                                          start=True, stop=True), "pe")
                    last_tt = nc.vector.tensor_tensor(af[:, c, :], af[:, c, :],
                                                      rb[:], op=ALU.mult)
                osb = p3.tile([128, 8, 512], F32, tag="osb", name="osb")
                last_act = last_mm = None
                for m in range(8):
                    po = ps_v.tile([128, 512], F32, tag="psv", name="po")
                    for c in range(8):
                        last_mm = nc.tensor.matmul(
                            po[:], wout_sb[:, c, ds(m * 128, 128)],
                            af[:, c, :], start=(c == 0), stop=(c == 7))
                    last_act = nc.scalar.activation(osb[:, m, :], po[:], AF.Identity,
                                                    bias=bout_sb[:, ds(m, 1)])
                    if m == 0:
                        gate(last_act, "s")
                nc.sync.dma_start(
                    outT_d.ap().rearrange("(m p) t -> p m t", p=128)[:, :, ts(half, 512)],
                    osb[:])
                return {"v": last_tt, "s": last_act, "pe": last_mm}

            # Emission order = engine-queue order. Interleave projection
            # t-blocks (PE-heavy) with even-q-block attention (ACT-heavy);
            # A2A half 0 overlaps odd-q-block attention; phase 3 of half 0
            # overlaps the A2A of half 1.
            for tb in range(4):
                emit_proj(tb)
            pi = 4
            for b in range(B):
                for j in (0, 2):
                    for h in range(HPC):
                        attention_unit(b, j, h)
                        if pi < NTB:
                            emit_proj(pi)
                            pi += 1
            stage_and_a2a(0)
            af0, rsrc0 = phase3_prefetch(0)  # loads run mid-pass-2, after cc0
            for b in range(B):
                for j in (1, 3):
                    for h in range(HPC):
                        le, lp, lc = attention_unit(b, j, h)
            stage_and_a2a(1)
            af1, rsrc1 = phase3_prefetch(1)
            lasts = phase3_compute(0, af0, rsrc0, {"v": lc, "s": le, "pe": lp})
            phase3_compute(1, af1, rsrc1, lasts)

    nc.compile()
    return nc


def prep_inputs(query, w_in, b_in, w_out, b_out):
    """Shard + lay out host-side. Returns in_maps for the 8 cores."""
    query = np.asarray(query, dtype=np.float32)
    w_in = np.asarray(w_in, dtype=np.float32)
    b_in = np.asarray(b_in, dtype=np.float32)
    w_out = np.asarray(w_out, dtype=np.float32)
    b_out = np.asarray(b_out, dtype=np.float32)

    qT = np.ascontiguousarray(query.reshape(T, E).T).astype(BF16)
    woutT = np.ascontiguousarray(w_out.T).astype(BF16)
    b_v = b_in[2 * E:3 * E]
    bout_eff = (b_out + w_out @ b_v).reshape(8, 128).T.copy()  # [128, 8]

    # causal masks for the 4 diagonal 128x512 blocks: mask[m][p, q] = p <= q-128m
    qidx = np.arange(512)[None, :]
    pidx = np.arange(128)[:, None]
    masks = np.stack([(pidx <= qidx - 128 * m) for m in range(4)], axis=1)
    masks = masks.astype(BF16)  # [128, 4, 512]

    sel = np.zeros((16, 8, 128), dtype=BF16)
    for c in range(8):
        sel[2 * c, c, 0:64] = 1.0
        sel[2 * c + 1, c, 64:128] = 1.0

    in_maps = []
    for c in range(N_CORES):
        r = slice(128 * c, 128 * c + 128)
        wqk = np.concatenate([w_in[:E][r].T, w_in[E:2 * E][r].T], axis=1)
        wv = w_in[2 * E:3 * E][r].T
        bqk = np.stack([b_in[:E][r], b_in[E:2 * E][r]], axis=1)
        in_maps.append({
            "qT": qT,
            "wqk": np.ascontiguousarray(wqk).astype(BF16),
            "wv": np.ascontiguousarray(wv).astype(BF16),
            "bqk": np.ascontiguousarray(bqk),
            "wout": woutT,
            "bout": np.ascontiguousarray(bout_eff),
            "masks": masks,
            "sel": sel,
        })
    return in_maps


def run_on_hw(in_maps, trace=False, **kw):
    from concourse.bass_utils import run_bass_kernel_spmd

    if "nc" not in _CACHE:
        _CACHE["nc"] = build_kernel()
    return run_bass_kernel_spmd(_CACHE["nc"], in_maps, list(range(N_CORES)),
                                trace=trace, **kw)


def kernel(query, w_in, b_in, w_out, b_out):
    in_maps = prep_inputs(query, w_in, b_in, w_out, b_out)
    res = run_on_hw(in_maps)
    parts = [res.results[c]["outT"].T for c in range(N_CORES)]  # [TL, E] each
    out = np.concatenate(parts, axis=0).reshape(B, S, E)
    return out.astype(np.float32)
